# revision 68
# baseline (speedup 1.0000x reference)
"""Causal multi-head attention block on 8 NeuronCores (Trainium2, Bass/Tile).

Reference computation (per batch b):
  Q = x @ W_Q + b_Q ; K = x @ W_K + b_K ; V = x @ W_V + b_V   (per head)
  scores = Q K^T / sqrt(H); causal mask; probs = softmax(scores)
  out = (probs @ V) @ W_O + b_O

Sharding: core c -> batch c//2, head-group c%2 (6 of 12 heads).
Each core computes a partial output [S, D] (its heads' contribution,
with b_Q/b_K applied on-device). Host sums the two head-group partials
per batch and adds b_O + sum_nh b_V[n,h] * W_O[n,h,:] (exact: the b_V
term factors out because softmax rows sum to 1).

Device-side layout choices (v3, bf16 + interleaved schedule):
  - all matmul operands are bf16 (same 1 cycle/row PE rate as fp32r but
    with no >=256 moving-dim constraint); PSUM accumulation stays fp32.
    DMA traffic halves; output is written bf16 and upcast on host.
  - scores are computed transposed ([k, q]); the softmax sum over k is
    taken by the PV matmul via a ones column on V.
  - the two heads of a pair share one 2-bank PSUM score tile
    [128, 2, 512]; a single Exp activation covers both heads. Score
    tiles are double-buffered so the PE can run a k-tile ahead of Exp.
  - the attention inner loop is Activation-paced (exp ~0.9us/k-tile vs
    ~0.64us of PE work), so PE filler work (s2=1 projections, qb0
    output-projection groups) is interleaved between k-tiles to keep
    the PE busy through the attention phases.
  - qb1 of the last pair streams its softmax normalize per 256-column
    half (columns [0:256] are final after k-tile 5), so the final
    output-projection tiles overlap the tail of attention instead of
    serializing after it.
  - engine balance: exp + qb0 bias-adds + dh0 out-copies on Activation;
    reciprocal/normalize + s2=1 bias-adds + dh1 out-copies on DVE;
    causal masks, V-copies, and broadcasts on gpsimd.
  - projections, warm-up, and output-projection accumulators share one
    double-buffered 1-bank PSUM pool (8 banks total in use).
"""

import sys

sys.path.insert(0, "/opt/trn_rl_repo")

from contextlib import ExitStack

import ml_dtypes
import numpy as np

import concourse.bass as bass
import concourse.tile as tile
from concourse import bacc, mybir
from concourse.bass_utils import run_bass_kernel_spmd

B, S, D, N, H = 4, 1024, 768, 12, 64
NHC = 6            # heads per core
NPAIR = NHC // 2   # head pairs per core (2 heads stacked -> 128 partitions)
HD = NHC * H       # 384: per-core packed head dim
P = 128
NDT = D // P       # 6 d-tiles
NST = S // P       # 8 s-tiles (also k-tiles)
QB = 512           # q block (moving-dim tile for most matmuls)
NQB = S // QB      # 2
F32 = mybir.dt.float32
BF16 = mybir.dt.bfloat16
FP8 = mybir.dt.float8e4
NDC = D // 256     # 3 DoubleRow d-chunks (256 contraction rows each)
W8_SCALE = 16.0    # host-side W_Q/W_K scale keeping fp8 out of subnormals
EXP_SCALE = 1.0 / np.sqrt(float(H))

_CACHE = {}


def _build():
    nc = bacc.Bacc()
    # x8|xr8, wq8|wk8, wv8|wvr8 concatenated host-side: fewer, larger DMAs
    # (each dma_start costs ~0.6us of serialized SP issue time).
    xt8_d = nc.declare_dram_parameter("xt8x", [2 * D, S], FP8, isOutput=False)
    # weights are host-packed per-partition-contiguous ([P, ...] flat rows):
    # 4.6KB runs per partition instead of 384B rows, which would pay the
    # sub-512B DMA descriptor penalty.
    wqk_d = nc.declare_dram_parameter("wqk8", [P, 2 * NDC * 2 * HD], FP8, isOutput=False)
    wvx_d = nc.declare_dram_parameter("wv8x", [P, 2 * NDC * 2 * HD], FP8, isOutput=False)
    wo_d = nc.declare_dram_parameter("wo", [HD, D], BF16, isOutput=False)
    bqk_d = nc.declare_dram_parameter("bqk", [P, 2 * NPAIR], F32, isOutput=False)
    tri_d = nc.declare_dram_parameter("trimask", [P, 2 * P], BF16, isOutput=False)
    out_d = nc.declare_dram_parameter("out", [S, D], BF16, isOutput=True)

    # fp8 path: DoubleRow matmuls contract 2 rows per partition, so x and
    # the W matrices are addressed as [p, chunk, i, *], d = c*256 + i*128 + p.
    # xr8/wvr8 are fp8 quantization residuals: adding their cross-terms into
    # the same PSUM group (same dequant scale) cancels most of the fp8 error.
    xt8_r = xt8_d[:].rearrange("(two c i p) s -> p two c i s", p=P, i=2, two=2)
    wqk_r = wqk_d[:]
    wvx_r = wvx_d[:]
    wo_r = wo_d[:].rearrange("(t p) d -> p t d", p=P)

    with tile.TileContext(nc) as tc, ExitStack() as ctx:
        consts = ctx.enter_context(tc.tile_pool(name="consts", bufs=1))
        persist = ctx.enter_context(tc.tile_pool(name="persist", bufs=1))
        etp = ctx.enter_context(tc.tile_pool(name="etp", bufs=4))
        smalls = ctx.enter_context(tc.tile_pool(name="smalls", bufs=4))
        outp = ctx.enter_context(tc.tile_pool(name="outp", bufs=3))

        x8x = consts.tile([P, 2, NDC, 2, S], FP8)
        x8 = x8x[:, 0]
        xr8 = x8x[:, 1]
        wqk = consts.tile([P, 2, NDC, 2, HD], FP8)
        wq_sb = wqk[:, 0]
        wk_sb = wqk[:, 1]
        wvx = consts.tile([P, 2, NDC, 2, HD], FP8)
        wv_sb = wvx[:, 0]
        wvr_sb = wvx[:, 1]
        bqk_sb = consts.tile([P, 2, NPAIR], F32)
        bq_sb = bqk_sb[:, 0]
        bk_sb = bqk_sb[:, 1]
        tri = consts.tile([P, 2, P], BF16)
        wo_sb = consts.tile([P, NPAIR, D], BF16)

        # ---- DMA emission order == priority order on the shared DMA device.
        # Q/K projections of all pairs run first (need only wqk8 + x qb0),
        # then V; later phases' tensors stream behind.
        nc.sync.dma_start(out=x8x[:, :, :, :, 0:QB], in_=xt8_r[:, :, :, :, 0:QB])
        nc.sync.dma_start(
            out=wqk, in_=wqk_r.rearrange("p (two c i h) -> p two c i h", two=2, c=NDC, i=2)
        )
        nc.sync.dma_start(out=bqk_sb, in_=bqk_d[:].rearrange("p (two g) -> p two g", two=2))
        nc.sync.dma_start(
            out=wvx, in_=wvx_r.rearrange("p (two c i h) -> p two c i h", two=2, c=NDC, i=2)
        )
        nc.sync.dma_start(
            out=tri, in_=tri_d[:].rearrange("p (two q) -> p two q", two=2)
        )
        nc.sync.dma_start(out=x8x[:, :, :, :, QB:S], in_=xt8_r[:, :, :, :, QB:S])
        nc.sync.dma_start(out=wo_sb, in_=wo_r)

        # ---- persistent activations ----
        qT = persist.tile([P, NPAIR, S], BF16)     # Q^T, head pairs stacked
        kT = persist.tile([P, NPAIR, S], BF16)
        # V + 64 ones columns per head: the PV matmul then emits the softmax
        # denominator replicated across 64 partitions (same moving-dim cost),
        # so the normalize needs no gpsimd partition_broadcast.
        vA = persist.tile([P, NST, NHC, 2 * H], BF16)
        zT = persist.tile([P, NPAIR, S], BF16)     # z^T (normalized), pairs stacked


        # Shared 1-bank accumulator pool: warm-up, Q/K/V projections, and
        # output-projection groups all round-robin its two buffers.
        ps_big = ctx.enter_context(tc.tile_pool(name="ps_big", bufs=2, space="PSUM"))
        # Score tiles: [128, 2, 512] fp32 = 2 banks each, double-buffered.
        ps_s = ctx.enter_context(tc.tile_pool(name="ps_s", bufs=2, space="PSUM"))
        # z accumulators (one per head of the active pair): 1 bank each.
        ps_z = ctx.enter_context(tc.tile_pool(name="ps_z", bufs=1, space="PSUM"))

        # PE warm-up: matmuls on a zeroed tile depend on no DMA, so they run
        # during the input-stream prologue and carry the PE clock (HAM) and
        # cost-model p-state ramp to full speed before the first real matmul.
        dums = consts.tile([P, QB], BF16)
        nc.gpsimd.memset(dums, 0.0)
        # Activation-table preload: the first table-based activation pays a
        # 1283ns ACT_TABLE_LOAD; trigger it at t=0 on a dummy tile so the
        # first bias-add (which gates the ps_big ring) doesn't.
        actwarm = consts.tile([1, 1], F32)
        nc.gpsimd.memset(actwarm, 0.0)
        nc.scalar.activation(
            actwarm, actwarm, mybir.ActivationFunctionType.Exp
        )
        nc.vector.memset(vA[:, :, :, H : 2 * H], 1.0)
        wps = ps_big.tile([P, QB], F32, name="warm", tag="big")
        for i in range(12):
            nc.tensor.matmul(
                wps,
                dums[:, 0:P],
                dums,
                start=(i == 0),
                stop=(i == 11),
            )

        def proj_begin(w_sb, g, s2, ps=None):
            """First fp8 DoubleRow set (x8 * w8) of a Q/K projection."""
            if ps is None:
                ps = ps_big.tile([P, QB], F32, tag="big")
            for c in range(NDC):
                nc.tensor.matmul(
                    ps,
                    w_sb[:, c, :, g * P : (g + 1) * P],
                    x8[:, c, :, s2 * QB : (s2 + 1) * QB],
                    start=(c == 0),
                    stop=False,
                    perf_mode=mybir.MatmulPerfMode.DoubleRow,
                )
            return ps

        def proj_end(ps, w_sb, b_sb, dst, g, s2, eng):
            """Second set (xr8 residual) + dequanting bias-add."""
            for c in range(NDC):
                nc.tensor.matmul(
                    ps,
                    w_sb[:, c, :, g * P : (g + 1) * P],
                    xr8[:, c, :, s2 * QB : (s2 + 1) * QB],
                    start=False,
                    stop=(c == NDC - 1),
                    perf_mode=mybir.MatmulPerfMode.DoubleRow,
                )
            dst_ap = dst[:, g, s2 * QB : (s2 + 1) * QB]
            if eng == "act":
                nc.scalar.activation(
                    dst_ap,
                    ps,
                    mybir.ActivationFunctionType.Identity,
                    bias=b_sb[:, g : g + 1],
                    scale=1.0 / W8_SCALE,
                )
            else:
                nc.vector.tensor_scalar(
                    dst_ap,
                    ps,
                    1.0 / W8_SCALE,
                    b_sb[:, g : g + 1],
                    mybir.AluOpType.mult,
                    mybir.AluOpType.add,
                )

        def proj_one(w_sb, b_sb, dst, g, s2, eng, ps=None):
            """Q/K projection of one head pair over one q-half: fp8 DoubleRow
            matmuls (256 contraction rows per instruction, 0.5 cycles/row),
            two sets (x8 + its fp8 residual) accumulated in one PSUM group.
            The bias-add applies the 1/W8_SCALE dequant."""
            ps = proj_begin(w_sb, g, s2, ps=ps)
            proj_end(ps, w_sb, b_sb, dst, g, s2, eng)

        def proj_v(st, copy_eng="dve"):
            """V projection of one k-tile: three fp8 DoubleRow sets
            (x8*wv8 + xr8*wv8 + x8*wvr8 — both residual cross-terms, same
            dequant scale) so V carries ~0.2% error despite fp8 operands."""
            vps = ps_big.tile([P, HD], F32, tag="big")
            sets = ((x8, wv_sb), (xr8, wv_sb), (x8, wvr_sb))
            for si, (xs, ws) in enumerate(sets):
                for c in range(NDC):
                    nc.tensor.matmul(
                        vps,
                        xs[:, c, :, st * P : (st + 1) * P],
                        ws[:, c, :, :],
                        start=(si == 0 and c == 0),
                        stop=(si == 2 and c == NDC - 1),
                        perf_mode=mybir.MatmulPerfMode.DoubleRow,
                    )
            if copy_eng == "act":
                nc.scalar.mul(
                    vA[:, st, :, 0:H],
                    vps.rearrange("p (n h) -> p n h", n=NHC),
                    1.0 / W8_SCALE,
                )
            else:
                nc.vector.tensor_scalar_mul(
                    vA[:, st, :, 0:H],
                    vps.rearrange("p (n h) -> p n h", n=NHC),
                    1.0 / W8_SCALE,
                )

        def norm_block(zzps, g, q0, c0, c1):
            """Normalize z columns [c0, c1) of pair g's block at q offset q0.
            The PV matmul already replicated the denominator over partitions
            H..2H, so this is just reciprocal + multiply on DVE."""
            w = c1 - c0
            for hh in range(2):
                hp = hh * H
                rb = smalls.tile([H, w], F32, tag="rb")
                nc.vector.reciprocal(rb, zzps[hh][H : 2 * H, c0:c1])
                nc.vector.tensor_mul(
                    zT[hp : hp + H, g, q0 + c0 : q0 + c1],
                    zzps[hh][0:H, c0:c1],
                    rb,
                )

        def attend_stream(blocks):
            """Run all (pair, q-block) attention blocks as ONE software-
            pipelined stream: the next k-tile's (exp-independent) score
            matmuls are emitted before each PV — across block boundaries
            too — so the in-order PE always has score work while Exp runs.
            Each block dict: g, qb, fillers (PE work units popped one per
            k-tile), post {kt: [fn(zzps)]}, norm (auto-normalize at end)."""
            seq = []
            for bi, b in enumerate(blocks):
                nkt = (b["qb"] + 1) * QB // P
                for kt in range(nkt):
                    seq.append((bi, kt, nkt))
            zz = [None] * len(blocks)

            def scores(bi, kt, nkt):
                b = blocks[bi]
                g, qb = b["g"], b["qb"]
                q0 = qb * QB
                o = max(kt * P - q0, 0)  # first live column
                sps = ps_s.tile([P, 2, QB], F32, tag="s")
                for hh in range(2):
                    hp = hh * H
                    nc.tensor.matmul(
                        sps[:, hh, o:QB],
                        kT[hp : hp + H, g, kt * P : (kt + 1) * P],
                        qT[hp : hp + H, g, q0 + o : q0 + QB],
                        start=True,
                        stop=True,
                        tile_position=(hp, 0),
                    )
                et = etp.tile([P, 2, QB], BF16)
                nc.scalar.activation(
                    et[:, :, o:QB],
                    sps[:, :, o:QB],
                    mybir.ActivationFunctionType.Exp,
                    scale=EXP_SCALE,
                )
                if kt * P - q0 >= -(P - 1):  # diagonal tile: partial block
                    # final k-tile's mask gates the block-end chain: run it
                    # on DVE (fast bf16 path) instead of gpsimd
                    eng = nc.vector if kt == nkt - 1 else nc.gpsimd
                    eng.tensor_mul(
                        et[:, :, o : o + P], et[:, :, o : o + P], tri
                    )
                return et, o

            pending = scores(*seq[0])
            for idx, (bi, kt, nkt) in enumerate(seq):
                b = blocks[bi]
                g, qb = b["g"], b["qb"]
                if zz[bi] is None:
                    zz[bi] = [
                        ps_z.tile([2 * H, QB], F32, name=f"zps{hh}", tag=f"z{hh}")
                        for hh in range(2)
                    ]
                if idx + 1 < len(seq):
                    nxt = scores(*seq[idx + 1])
                et, o = pending
                for hh in range(2):
                    nc.tensor.matmul(
                        zz[bi][hh][:, o:QB],
                        vA[:, kt, 2 * g + hh, :],
                        et[:, hh, o:QB],
                        start=(kt == 0),
                        stop=(kt == nkt - 1),
                    )
                if idx + 1 < len(seq):
                    pending = nxt
                post = b.get("post")
                if post and kt in post:
                    for fn in post[kt]:
                        fn(zz[bi])
                fillers = b.get("fillers")
                if fillers:
                    fillers.pop(0)()
                if kt == nkt - 1 and b.get("norm", True):
                    norm_block(zz[bi], g, qb * QB, 0, QB)

        def out_group(row0, dh, out_t, copy_eng="dve", ops=None):
            """One output-projection accumulation group: rows [row0, row0+P),
            column half dh."""
            if ops is None:
                ops = ps_big.tile([P, D // 2], F32, tag="big")
            for g in range(NPAIR):
                nc.tensor.matmul(
                    ops,
                    zT[:, g, row0 : row0 + P],
                    wo_sb[:, g, dh * (D // 2) : (dh + 1) * (D // 2)],
                    start=(g == 0),
                    stop=(g == NPAIR - 1),
                )
            dst = out_t[:, dh * (D // 2) : (dh + 1) * (D // 2)]
            if copy_eng == "act":
                nc.scalar.copy(dst, ops)
            else:
                nc.vector.tensor_copy(out=dst, in_=ops)

        def out_tile(row0, tail=False, last=False):
            """Full output tile rows [row0, row0+P): both dh groups, one DMA.
            Tiles emitted after the last Exp (`tail`) borrow the (drained)
            score pool for their accumulators and copy on the (free) Act
            engine; the `last` tile splits its copies Act/DVE so the final
            serial chain is shortest."""
            out_t = outp.tile([P, D], BF16)
            if tail or last:
                ops2 = ps_s.tile([P, 2, QB], F32, tag="s")
                for dh in range(2):
                    for g in range(NPAIR):
                        nc.tensor.matmul(
                            ops2[:, dh, 0 : D // 2],
                            zT[:, g, row0 : row0 + P],
                            wo_sb[:, g, dh * (D // 2) : (dh + 1) * (D // 2)],
                            start=(g == 0),
                            stop=(g == NPAIR - 1),
                        )
                if last:
                    nc.scalar.copy(out_t[:, 0 : D // 2], ops2[:, 0, 0 : D // 2])
                    nc.vector.tensor_copy(
                        out=out_t[:, D // 2 : D], in_=ops2[:, 1, 0 : D // 2]
                    )
                else:
                    nc.scalar.copy(
                        out_t.rearrange("p (dh d) -> p dh d", dh=2),
                        ops2[:, :, 0 : D // 2],
                    )
            else:
                out_group(row0, 0, out_t)
                out_group(row0, 1, out_t)
            nc.sync.dma_start(out=out_d[row0 : row0 + P, :], in_=out_t)

        # ---- schedule: all qb0 Q/K projections first (cheap fp8 DoubleRow,
        # need only wq8/wk8 + x8 qb0); V projections and qb0 output tiles
        # ride as fillers inside the Activation-paced attention loops so the
        # PE never idles on Exp.
        # The score pool is idle until the first attention block: lend its 4
        # banks to four of the six qb0 projections so none of them waits on
        # the ps_big ring (whose slot reuse is gated by the serial bias-adds).
        psl = [
            ps_s.tile([P, 2, QB], F32, name=f"psl{i}", tag="s") for i in range(2)
        ]
        proj_one(wq_sb, bq_sb, qT, 0, 0, "act")
        proj_one(wq_sb, bq_sb, qT, 1, 0, "dve", ps=psl[0][:, 0])
        proj_one(wq_sb, bq_sb, qT, 2, 0, "act", ps=psl[0][:, 1])
        proj_one(wk_sb, bk_sb, kT, 0, 0, "dve", ps=psl[1][:, 0])
        proj_one(wk_sb, bk_sb, kT, 1, 0, "act", ps=psl[1][:, 1])
        proj_one(wk_sb, bk_sb, kT, 2, 0, "dve")
        proj_v(0)

        # One continuous attention stream over all (pair, q-block) blocks.
        # Fillers place the remaining projections and the qb0 output tiles
        # inside the Exp-paced qb1 windows, where the PE otherwise idles;
        # the last block streams its normalize per 128-column block (final
        # after k-tile 4+qt) so qb1 output tiles overlap the attention tail.
        attend_stream([
            dict(g=0, qb=0, fillers=[
                lambda: proj_v(1, "act"),
                lambda: proj_v(2, "act"),
                lambda: proj_v(3, "act"),
            ]),
            dict(g=1, qb=0, fillers=[
                lambda: proj_one(wq_sb, bq_sb, qT, 0, 1, "dve"),
                lambda: proj_one(wk_sb, bk_sb, kT, 0, 1, "dve"),
            ]),
            dict(g=2, qb=0, fillers=[
                lambda: proj_v(4),
                lambda: proj_v(5),
            ]),
            dict(g=0, qb=1, fillers=[
                lambda: proj_v(6),
                lambda: proj_v(7),
                lambda: proj_one(wq_sb, bq_sb, qT, 1, 1, "dve"),
                lambda: proj_one(wk_sb, bk_sb, kT, 1, 1, "dve"),
                lambda: proj_one(wq_sb, bq_sb, qT, 2, 1, "dve"),
                lambda: proj_one(wk_sb, bk_sb, kT, 2, 1, "dve"),
            ]),
            dict(g=1, qb=1, fillers=[
                lambda: out_tile(0 * P),
                lambda: out_tile(1 * P),
                lambda: out_tile(2 * P),
            ]),
            dict(g=2, qb=1, norm=False, fillers=[
                lambda: out_tile(3 * P),
            ], post={
                4: [lambda zz: norm_block(zz, 2, QB, 0, P)],
                5: [
                    lambda zz: norm_block(zz, 2, QB, P, 2 * P),
                    lambda zz: out_tile(QB + 0 * P),
                ],
                6: [
                    lambda zz: norm_block(zz, 2, QB, 2 * P, 3 * P),
                    lambda zz: out_tile(QB + 1 * P, tail=True),
                ],
                7: [
                    lambda zz: norm_block(zz, 2, QB, 3 * P, QB),
                    lambda zz: out_tile(QB + 2 * P, tail=True),
                ],
            }),
        ])
        out_tile(QB + 3 * P, tail=True)

    if not nc.is_finalized():
        nc.finalize()
    return nc


def _get_program():
    if "nc" not in _CACHE:
        _CACHE["nc"] = _build()
    return _CACHE["nc"]


def make_in_maps(
    normalized_resid_pre, W_Q, W_K, W_V, W_O, b_Q, b_K, b_V=None, b_O=None, **_unused
):
    bf = ml_dtypes.bfloat16
    f8 = ml_dtypes.float8_e4m3
    x = np.asarray(normalized_resid_pre, np.float32)
    W_Q, W_K, W_V = (np.asarray(a, np.float32) for a in (W_Q, W_K, W_V))
    W_O = np.asarray(W_O, np.float32)
    b_Q, b_K = np.asarray(b_Q, np.float32), np.asarray(b_K, np.float32)

    tri = np.triu(np.ones((P, P), np.float32))
    tri2 = np.concatenate([tri, tri], axis=1).astype(bf)

    def perm(a):
        """[D, HD] -> per-partition-contiguous [P, NDC*2*HD] in the
        DoubleRow (c, i) layout (d = c*256 + i*128 + p)."""
        return a.reshape(NDC, 2, P, HD).transpose(2, 0, 1, 3).reshape(P, -1)

    def pack_w1(w):
        """fp8 value part only (scaled by W8_SCALE)."""
        return perm((w * W8_SCALE).astype(f8))

    def pack_w2(w):
        """fp8 value + residual, concatenated along the free dim."""
        ws = w * W8_SCALE
        w8 = ws.astype(f8)
        wr8 = (ws - w8.astype(np.float32).astype(np.float32)).astype(f8)
        return np.concatenate([perm(w8), perm(wr8)], axis=1)

    in_maps = []
    for c in range(8):
        b, hg = divmod(c, 2)
        hs = slice(hg * NHC, (hg + 1) * NHC)
        xt = x[b].T
        xt8 = xt.astype(f8)
        xtr8 = (xt - xt8.astype(np.float32)).astype(f8)
        in_maps.append(
            {
                "xt8x": np.ascontiguousarray(np.concatenate([xt8, xtr8], 0)),
                "wqk8": np.ascontiguousarray(
                    np.concatenate(
                        [
                            pack_w1(W_Q[hs].transpose(1, 0, 2).reshape(D, HD)),
                            pack_w1(W_K[hs].transpose(1, 0, 2).reshape(D, HD)),
                        ],
                        axis=1,
                    )
                ),
                "wv8x": np.ascontiguousarray(
                    pack_w2(W_V[hs].transpose(1, 0, 2).reshape(D, HD))
                ),
                "wo": np.ascontiguousarray(W_O[hs].reshape(HD, D).astype(bf)),
                "bqk": np.ascontiguousarray(
                    np.concatenate(
                        [b_Q[hs].reshape(NPAIR, P).T, b_K[hs].reshape(NPAIR, P).T],
                        axis=1,
                    )
                ),
                "trimask": tri2,
            }
        )
    return in_maps


def kernel(
    normalized_resid_pre, W_Q, W_K, W_V, W_O, b_Q, b_K, b_V, b_O, **_unused
):
    W_O = np.asarray(W_O, np.float32)
    b_V, b_O = np.asarray(b_V, np.float32), np.asarray(b_O, np.float32)
    in_maps = make_in_maps(
        normalized_resid_pre, W_Q, W_K, W_V, W_O, b_Q, b_K
    )

    nc = _get_program()
    res = run_bass_kernel_spmd(nc, in_maps, list(range(8))).results

    out = np.zeros((B, S, D), np.float32)
    for c in range(8):
        out[c // 2] += np.asarray(res[c]["out"], dtype=np.float32)
    out += b_O + np.einsum("nh,nhd->d", b_V, W_O)
    return out
